# revision 1
# baseline (speedup 1.0000x reference)
"""PSENet-style OHEM + dice loss on 8 Trainium2 NeuronCores.

Data-parallel over the batch: core b processes image b entirely on-chip
(one pass over its 24.5 MB of inputs; the kernel is HBM-bandwidth bound).
Each core emits 22 partial sums; the final scalar means are combined on
the host (they are 8x22 floats - no collective needed).

Math notes (labels / masks are exactly 0.0/1.0):
  dice(x, g, M) needs  a = sum(sig(x)*g*M), b = sum(sig(x)^2*M), c = sum(g*M).
  - masked logits: xm = min(x, (2M-1)*BIG)  =>  sig(xm) = sig(x)*M (up to
    sig(-BIG) ~ 2e-22, far below fp32 noise on these sums). One DVE
    scalar_tensor_tensor (STT) pass.
  - a: DVE STT (g*1.0)*sig with accum_out - product + sum in one pass.
  - b: ACT Square with accum_out (sum of squares) in one pass.
  - c: sum(g*M): GPSIMD tensor_mul product + ACT Copy/accum reduce for the
    six kernel channels (keeps DVE free); one DVE STT+accum for the text
    channel. (tensor_tensor_reduce would do it in one DVE pass but crashes
    the device - NRT_EXEC_UNIT_UNRECOVERABLE - so it is avoided.)
  - accum_out columns land in [128,16] per-engine accumulators; one
    [128,16]x[128,1] ones-matmul per accumulator does the cross-partition
    reduction; host combines 8x32 floats into the final three scalars.
  - The last kernel channel is processed as two half-tiles so the compute
    tail after the final DMA is halved (single-shot latency).
  - OHEM: for these inputs 3*pos_num >= total_neg for every image, so the
    OHEM threshold is the minimum negative score and the selected mask is
    exactly the training mask. The host VERIFIES the sufficient condition
    (RATIO+1)*pos_num >= N (pos_num = text-channel c; since sum_g >= pos_num
    this implies RATIO*pos_num >= total_neg, ~28 sigma of margin here) and
    falls back to a full host reference if it ever failed.

Single-shot latency tuning (cost-model trace verified): x6/m/g6 DMAs are
issued first and the text xm+sigmoid run before the M/Mb mask builds, so
ACT starts ~4 us earlier; x/g pools use 3 buffers so the DMA stream never
stalls on the head compute chain (was an 8.7 us gap); the split last
channel keeps the post-final-DMA tail short.

Measured on 8 axon-tunneled trn2 cores: steady-state ~62 us/image at the
~24.5 MB / ~400 GB/s-per-core HBM roofline (DMA busy 68 us modeled with
zero mid-stream gaps; engines below it: ACT ~63, DVE ~61, GPSIMD ~33 us).
Cost-model single-shot estimate 88 us (was 95.6 before the reorder).
"""

import os
import sys

import numpy as np

for _p in ("/opt/trn_rl_repo", "/root/.axon_site/_ro/trn_rl_repo"):
    if os.path.isdir(_p) and _p not in sys.path:
        sys.path.append(_p)

import concourse.bacc as bacc
import concourse.tile as tile
from concourse import mybir
from concourse.bass_utils import run_bass_kernel_spmd

B, C, H, W = 8, 7, 640, 640
NK = C - 1            # kernel channels
N = H * W             # pixels per image
P = 128               # SBUF partitions
F = N // P            # free dim per tile (3200)
BIG = 50.0
NCORES = 8
LAMBDA = 0.7
RATIO = 3

_dt = mybir.dt.float32
_AF = mybir.ActivationFunctionType
_ALU = mybir.AluOpType


def _img_ap(dram_ap):
    """[H, W] dram slab -> [128, 3200] partition-major access pattern."""
    return dram_ap.rearrange("(p q) w -> p (q w)", p=P)


def build_nc(debug=False, reps=1):
    nc = bacc.Bacc("TRN2", target_bir_lowering=False, debug=debug)
    x_d = nc.dram_tensor("x", [C, H, W], _dt, kind="ExternalInput")
    g_d = nc.dram_tensor("g", [C, H, W], _dt, kind="ExternalInput")
    m_d = nc.dram_tensor("m", [H, W], _dt, kind="ExternalInput")
    res_d = nc.dram_tensor("res", [16, 2], _dt, kind="ExternalOutput")

    with (
        tile.TileContext(nc) as tc,
        tc.tile_pool(name="const", bufs=1) as cpool,
        tc.tile_pool(name="xin", bufs=3) as xpool,
        tc.tile_pool(name="gin", bufs=3) as gpool,
        tc.tile_pool(name="xmp", bufs=2) as xmpool,
        tc.tile_pool(name="sbp", bufs=2) as sbpool,
        tc.tile_pool(name="junk", bufs=2) as jpool,
        tc.tile_pool(name="ps", bufs=1, space="PSUM") as ppool,
    ):
        # accum_out columns; res col 0 = partition-sums of acc_dve,
        # res col 1 = partition-sums of acc_act (see _combine for layout)
        acc_dve = cpool.tile([P, 16], _dt)
        acc_act = cpool.tile([P, 16], _dt)
        ones_t = cpool.tile([P, 1], _dt)
        nc.gpsimd.memset(ones_t[:], 1.0)
        nc.vector.memset(acc_dve[:], 0.0)
        nc.scalar.memzero(acc_act[:])

        def image_body(rep):
            # ---- text channel (ch 6) first: its logits gate everything ----
            xt6 = xpool.tile([P, F], _dt, tag="xin", name=f"xt6_r{rep}")
            nc.sync.dma_start(xt6[:], _img_ap(x_d.ap()[C - 1]))
            m_t = cpool.tile([P, F], _dt, tag="m_t", name=f"m_t_r{rep}")
            nc.sync.dma_start(m_t[:], _img_ap(m_d.ap()))
            gt6 = gpool.tile([P, F], _dt, tag="gin", name=f"gt6_r{rep}")
            nc.sync.dma_start(gt6[:], _img_ap(g_d.ap()[C - 1]))


            # kernel-channel mask M = (x_text > 0) * m, and (2M-1)*BIG.
            # Emitted after the text xm/sigma (see channel_body call below)
            # would be ideal, but M/Mb only gate kernel channels; keep them
            # after mb so the text sigmoid starts as early as possible.
            M_t = cpool.tile([P, F], _dt, tag="M_t", name=f"M_t_r{rep}")
            Mb_t = cpool.tile([P, F], _dt, tag="Mb_t", name=f"Mb_t_r{rep}")


            def channel_body(xt, gt, maskb, msel, col):
                # xm = min(x, maskb)   (DVE)
                xm = xmpool.tile([P, F], _dt, tag="xmp", name=f"xm{col}_r{rep}")
                nc.vector.scalar_tensor_tensor(
                    xm[:], xt[:], 0.0, maskb[:], _ALU.add, _ALU.min
                )
                # sb = sigmoid(xm)     (ACT)
                sbt = sbpool.tile([P, F], _dt, tag="sbp", name=f"sb{col}_r{rep}")
                nc.scalar.activation(sbt[:], xm[:], _AF.Sigmoid)
                # b = sum(sb^2)        (ACT, overwrite xm with junk output)
                nc.scalar.activation(
                    xm[:], sbt[:], _AF.Square,
                    accum_out=acc_act[:, col:col + 1],
                )
                # a = sum(g * sb)      (DVE STT w/ accum, overwrite xt)
                nc.vector.scalar_tensor_tensor(
                    xt[:], gt[:], 1.0, sbt[:], _ALU.mult, _ALU.mult,
                    accum_out=acc_dve[:, col:col + 1],
                )
                # c = sum(g * msel): GPSIMD product + ACT Copy/accum reduce
                jg = jpool.tile([P, F], _dt, tag="junk", name=f"jg{col}_r{rep}")
                nc.gpsimd.tensor_mul(jg[:], gt[:], msel[:])
                nc.scalar.activation(
                    xm[:], jg[:], _AF.Copy,
                    accum_out=acc_act[:, 8 + col:9 + col],
                )


            mb_t = cpool.tile([P, F], _dt, tag="mb_t", name=f"mb_t_r{rep}")
            nc.vector.tensor_scalar(
                mb_t[:], m_t[:], 2.0 * BIG, -BIG, _ALU.mult, _ALU.add
            )
            # text xm + sigma first (only needs x6 + mb); then M/Mb
            xm6 = xmpool.tile([P, F], _dt, tag="xmp", name=f"xm6_r{rep}")
            sb6 = sbpool.tile([P, F], _dt, tag="sbp", name=f"sb6_r{rep}")
            nc.vector.scalar_tensor_tensor(
                xm6[:], xt6[:], 0.0, mb_t[:], _ALU.add, _ALU.min
            )
            nc.scalar.activation(sb6[:], xm6[:], _AF.Sigmoid)
            nc.vector.scalar_tensor_tensor(
                M_t[:], xt6[:], 0.0, m_t[:], _ALU.is_gt, _ALU.mult
            )
            nc.vector.tensor_scalar(
                Mb_t[:], M_t[:], 2.0 * BIG, -BIG, _ALU.mult, _ALU.add
            )
            nc.scalar.activation(
                xm6[:], sb6[:], _AF.Square,
                accum_out=acc_act[:, 0:1]
            )
            nc.vector.scalar_tensor_tensor(
                xt6[:], gt6[:], 1.0, sb6[:], _ALU.mult, _ALU.mult,
                accum_out=acc_dve[:, 0:1],
            )
            nc.vector.scalar_tensor_tensor(
                xt6[:], gt6[:], 1.0, m_t[:], _ALU.mult, _ALU.mult,
                accum_out=acc_dve[:, 8:9],
            )


            for k in range(NK - 1):
                xt = xpool.tile([P, F], _dt, tag="xin", name=f"xk{k}_r{rep}")
                nc.sync.dma_start(xt[:], _img_ap(x_d.ap()[k]))
                gt = gpool.tile([P, F], _dt, tag="gin", name=f"gk{k}_r{rep}")
                nc.sync.dma_start(gt[:], _img_ap(g_d.ap()[k]))
                channel_body(xt, gt, Mb_t, M_t, 1 + k)

            # last kernel channel in two half-tiles so the post-final-DMA
            # compute tail is half as long (single-shot latency)
            k = NK - 1
            Fh = F // 2
            xt = xpool.tile([P, F], _dt, tag="xin", name=f"xk{k}_r{rep}")
            gt = gpool.tile([P, F], _dt, tag="gin", name=f"gk{k}_r{rep}")
            for h, (acol, bcol, ccol) in enumerate(((NK, NK, 14), (7, 8, 15))):
                xs = xt[:, h * Fh:(h + 1) * Fh]
                gs = gt[:, h * Fh:(h + 1) * Fh]
                src = _img_ap(x_d.ap()[k])
                nc.sync.dma_start(xs, src[:, h * Fh:(h + 1) * Fh])
                srcg = _img_ap(g_d.ap()[k])
                nc.sync.dma_start(gs, srcg[:, h * Fh:(h + 1) * Fh])
                xm = xmpool.tile([P, F], _dt, tag="xmp", name=f"xmL{h}_r{rep}")
                nc.vector.scalar_tensor_tensor(
                    xm[:, :Fh], xs, 0.0, Mb_t[:, h * Fh:(h + 1) * Fh],
                    _ALU.add, _ALU.min
                )
                sbt = sbpool.tile([P, F], _dt, tag="sbp", name=f"sbL{h}_r{rep}")
                nc.scalar.activation(sbt[:, :Fh], xm[:, :Fh], _AF.Sigmoid)
                nc.scalar.activation(
                    xm[:, :Fh], sbt[:, :Fh], _AF.Square,
                    accum_out=acc_act[:, bcol:bcol + 1],
                )
                nc.vector.scalar_tensor_tensor(
                    xm[:, Fh:2 * Fh], gs, 1.0, sbt[:, :Fh],
                    _ALU.mult, _ALU.mult,
                    accum_out=acc_dve[:, acol:acol + 1],
                )
                nc.vector.scalar_tensor_tensor(
                    sbt[:, Fh:2 * Fh], gs, 1.0,
                    M_t[:, h * Fh:(h + 1) * Fh], _ALU.mult, _ALU.mult,
                    accum_out=acc_dve[:, ccol:ccol + 1],
                )

        for rep in range(reps):
            image_body(rep)

        # cross-partition reduction of all accumulators with one ones-vector
        # matmul per accumulator: res row i <- sum_p acc[p, i]
        pr = ppool.tile([16, 2], _dt, tag="pr")
        nc.tensor.matmul(pr[:, 0:1], lhsT=acc_dve[:], rhs=ones_t[:],
                         start=True, stop=True)
        nc.tensor.matmul(pr[:, 1:2], lhsT=acc_act[:], rhs=ones_t[:],
                         start=True, stop=True)
        res_sb = cpool.tile([16, 2], _dt)
        nc.scalar.copy(res_sb[:], pr[:])
        nc.sync.dma_start(res_d.ap(), res_sb[:])

    nc.compile()
    return nc


_CACHE = {}


def _get_nc():
    if "nc" not in _CACHE:
        _CACHE["nc"] = build_nc(debug=False)
    return _CACHE["nc"]


def _combine(res_list):
    """res_list: per-image [16, 2] device sums -> (loss_text, loss_kernels, loss).

    Returns None if the OHEM fast-path precondition fails for any image.
    """
    lt_b = np.zeros(B, np.float64)
    lk_b = np.zeros(B, np.float64)
    for b in range(B):
        v = np.asarray(res_list[b], np.float64)
        a_t, b_t = v[0, 0], v[0, 1]
        c_t = v[8, 0]            # text c is the DVE accumulator slot
        pos_num = c_t                    # sum(gt_text * m), exact integer
        # sel == m iff pos_num == 0 (fallback) or neg_num == total_neg,
        # i.e. RATIO*pos_num >= total_neg = N - sum_g. Since sum_g >=
        # sum_g*m = pos_num, (RATIO+1)*pos_num >= N is sufficient and
        # avoids computing sum_g on device (~28 sigma of margin here).
        if not (pos_num == 0 or (RATIO + 1) * pos_num >= N):
            return None
        lt_b[b] = 1.0 - 2.0 * a_t / (b_t + 0.001 + c_t + 0.001)
        lk = 0.0
        for k in range(NK):
            a_k, b_k = v[1 + k, 0], v[1 + k, 1]
            c_k = v[9 + k, 0] + v[9 + k, 1]
            if k == NK - 1:  # second half of the split last channel
                a_k += v[7, 0]
                b_k += v[8, 1]
                c_k = v[14, 0] + v[15, 0]
            lk += 1.0 - 2.0 * a_k / (b_k + 0.001 + c_k + 0.001)
        lk_b[b] = lk / NK
    lt = np.float32(lt_b.mean())
    lk = np.float32(lk_b.mean())
    loss = np.float32(LAMBDA) * lt + np.float32(1.0 - LAMBDA) * lk
    return (lt, lk, np.float32(loss))


def _numpy_reference(outputs, labels, training_masks):
    """Full-fidelity host fallback (mirrors the original loss exactly)."""
    def sigmoid(z):
        return 1.0 / (1.0 + np.exp(-z, dtype=np.float64))

    texts = outputs[:, -1].reshape(B, N).astype(np.float64)
    kernels = outputs[:, :-1].reshape(B, NK, N).astype(np.float64)
    gt_texts = labels[:, -1].reshape(B, N).astype(np.float64)
    gt_kernels = labels[:, :-1].reshape(B, NK, N).astype(np.float64)
    tm = training_masks.reshape(B, N).astype(np.float64)

    pos = gt_texts > 0.5
    pos_num = np.sum(pos & (tm > 0.5), axis=1)
    neg = ~pos
    total_neg = np.sum(neg, axis=1)
    neg_num = np.minimum(pos_num * RATIO, total_neg)
    neg_scores = np.where(neg, texts, -np.inf)
    sorted_desc = -np.sort(-neg_scores, axis=1)
    idx = np.clip(neg_num - 1, 0, N - 1)
    thr = np.take_along_axis(sorted_desc, idx[:, None], axis=1)
    sel = (((texts >= thr) | pos) & (tm > 0.5)).astype(np.float64)
    fallback = (pos_num == 0) | (neg_num == 0)
    sel = np.where(fallback[:, None], tm, sel)

    def dice(inp, target, mask):
        p = sigmoid(inp) * mask
        t = target * mask
        a = np.sum(p * t, axis=-1)
        bb = np.sum(p * p, axis=-1) + 0.001
        cc = np.sum(t * t, axis=-1) + 0.001
        return 1.0 - 2.0 * a / (bb + cc)

    loss_text = dice(texts, gt_texts, sel).mean()
    sel_k = ((sigmoid(texts) > 0.5) & (tm > 0.5)).astype(np.float64)
    loss_kernels = dice(kernels, gt_kernels, sel_k[:, None, :]).mean(axis=1).mean()
    loss = LAMBDA * loss_text + (1.0 - LAMBDA) * loss_kernels
    return (np.float32(loss_text), np.float32(loss_kernels), np.float32(loss))


def kernel(outputs, labels, training_masks):
    outputs = np.asarray(outputs, dtype=np.float32)
    labels = np.asarray(labels, dtype=np.float32)
    training_masks = np.asarray(training_masks, dtype=np.float32)
    assert outputs.shape == (B, C, H, W)

    nc = _get_nc()
    in_maps = [
        {
            "x": np.ascontiguousarray(outputs[b]),
            "g": np.ascontiguousarray(labels[b]),
            "m": np.ascontiguousarray(training_masks[b]),
        }
        for b in range(B)
    ]
    r = None
    for attempt in range(3):
        try:
            r = run_bass_kernel_spmd(
                nc, in_maps, list(range(NCORES)),
                trace=_CACHE.get("trace", False),
            )
            break
        except Exception:
            if attempt == 2:
                raise
            _CACHE.pop("nc", None)
            nc = _get_nc()
    _CACHE["last_result"] = r
    res_list = [r.results[b]["res"] for b in range(B)]
    out = _combine(res_list)
    if out is None:
        # OHEM threshold is not the minimum negative score -> exact host path
        out = _numpy_reference(outputs, labels, training_masks)
    return out



# revision 27
# speedup vs baseline: 7.0790x; 7.0790x over previous
"""PSENet-style OHEM + dice loss on 8 Trainium2 NeuronCores — bf16 edition.

Data-parallel over the batch: core b processes image b entirely on-chip.
All inputs are cast to bf16 on the host (labels/masks are exactly 0/1 so
they are lossless; logits lose ~0.4% relative, far below the fp32 noise
floor of the 400k-element dice sums). This halves HBM traffic per image
from 24.5 MB to 12.25 MB (the fp32 kernel was HBM-bound at ~358 GB/s/core)
and doubles DVE tensor_tensor throughput (bf16 2x_1p mode).

DVE perf-mode reality (cost-model + HW): scalar_tensor_tensor has NO fast
uops (always 1x), tensor_tensor runs 2x with bf16, tensor_scalar/copy 4x.
So products are computed with plain TT at 2x and the *summations* are
farmed out instead of fused:

  per kernel channel k (mask M = (x_text>0)*m, Mb = (2M-1)*BIG):
    xm = min(x, Mb)        DVE TT            (sig(xm) = sig(x)*M)
    sb = sigmoid(xm)       ACT
    b  = sum sb^2          ACT Square+accum_out (k=0..3, fused, free)
                           k=4: GPSIMD product + PE column-sum
                           k=5: DVE TT half-products + TS copy-accum (4x)
    ap = g*sb              DVE TT -> PE column-sum  (a = sum ap)
    cp = g*M               GPSIMD (k=0..3) / DVE TT (k=4,5) -> PE column-sum
  text channel: same with mask m; pos_num = c_t doubles as the OHEM
  fast-path witness.

PE column sums: product tiles are padded to 3584 = 7x512 (pool buffers'
pad columns zeroed once), and a selector lhsT (ones in column k) routes
channel k into row k of a base-0 PSUM tile — matmul outputs must start at
partition 0/32/64. Each of the three PSUM regions (c @ cols 0:512,
a @ 512:1024, b @ 1024:1536) is one accumulation group spanning the whole
NEFF (start on first use of rep 0, stop on last use of the last rep), so
the per-image steady-state pays no reduction tail; three DVE tensor_scalar
copy-accums collapse the regions to [8,3] once at the end.

OHEM: for these inputs 3*pos_num >= total_neg for every image, so the
selected mask is exactly the training mask. The host VERIFIES
(RATIO+1)*pos_num >= N (pos_num = c_t, exact: 0/1 products, fp32 PSUM)
and falls back to a full host reference if it ever fails.

Engine budget per image (bf16, F=3200, cost-model): DVE ~34.9us,
ACT ~34.4us, GPSIMD ~32.2us (5 products), PE ~22us, DMA ~34.1us.
"""

import os
import sys

import numpy as np

for _p in ("/opt/trn_rl_repo", "/root/.axon_site/_ro/trn_rl_repo"):
    if os.path.isdir(_p) and _p not in sys.path:
        sys.path.append(_p)

import concourse.bacc as bacc
import concourse.tile as tile
from concourse import mybir
from concourse.bass_utils import run_bass_kernel_spmd

B, C, H, W = 8, 7, 640, 640
NK = C - 1            # kernel channels
N = H * W             # pixels per image
P = 128               # SBUF partitions
F = N // P            # free dim per tile (3200)
FP = 3584             # padded free dim (7 x 512) for PE column sums
BIG = 50.0
NCORES = 8
LAMBDA = 0.7
RATIO = 3

_dt = mybir.dt.bfloat16
_f32 = mybir.dt.float32
_AF = mybir.ActivationFunctionType
_ALU = mybir.AluOpType

# PSUM region base columns
_RC, _RA, _RB = 0, 512, 1024

_NP_BF16 = None


def _np_bf16():
    global _NP_BF16
    if _NP_BF16 is None:
        _NP_BF16 = mybir.dt.np(mybir.dt.bfloat16)
    return _NP_BF16


def _img_ap(dram_ap):
    """[H, W] dram slab -> [128, 3200] partition-major access pattern."""
    return dram_ap.rearrange("(p q) w -> p (q w)", p=P)


def build_nc(debug=False, reps=1, serial=False):
    nc = bacc.Bacc("TRN2", target_bir_lowering=False, debug=debug)
    x_d = nc.dram_tensor("x", [C, H, W], _dt, kind="ExternalInput")
    g_d = nc.dram_tensor("g", [C, H, W], _dt, kind="ExternalInput")
    m_d = nc.dram_tensor("m", [H, W], _dt, kind="ExternalInput")
    res_d = nc.dram_tensor("res", [16, 2], _f32, kind="ExternalOutput")
    res2_d = nc.dram_tensor("res2", [8, 2], _f32, kind="ExternalOutput")

    with (
        tile.TileContext(nc) as tc,
        tc.tile_pool(name="const", bufs=1) as cpool,
        tc.tile_pool(name="mask", bufs=2) as mkpool,
        tc.tile_pool(name="xin", bufs=5) as xpool,
        tc.tile_pool(name="gin", bufs=5) as gpool,
        tc.tile_pool(name="sbp", bufs=4) as sbpool,
        tc.tile_pool(name="apr", bufs=3) as apool,
        tc.tile_pool(name="gmp", bufs=3) as gmpool,
        tc.tile_pool(name="ps", bufs=1, space="PSUM") as ppool,
    ):
        # fp32 per-partition accumulators (fused accum_out targets).
        # acc_dve columns: 10,11 = b_5 halves (TS copy-accum)
        # acc_act columns: 0 = b_t, 1..4 = b_k (k=0..3)  (ACT Square accum)
        acc_dve = cpool.tile([P, 16], _f32)
        acc_act = cpool.tile([P, 16], _f32)
        ones_f = cpool.tile([P, 1], _f32)
        nc.gpsimd.memset(ones_f[:], 1.0)
        nc.vector.memset(acc_dve[:], 0.0)
        nc.scalar.memzero(acc_act[:])

        # PSUM column-sum accumulator. Channel k is routed to row k of a
        # base-0 [8,...] tile via a selector lhsT (ones in column k): rows
        # != k accumulate += 0. Regions: c @ 0:512, a @ 512:1024; each is
        # ONE accumulation group spanning the whole NEFF. (Channel-5 and
        # b_4 sums use DVE TS copy-accums into acc_dve instead, so the
        # post-final-DMA single-shot tail avoids the cold-PE colsum path.)
        cpsum = ppool.tile([8, 1024], _f32, tag="cps")
        res2_sb = cpool.tile([8, 2], _f32)
        nc.vector.memset(res2_sb[:], 0.0)
        rjunk = cpool.tile([8, 512], _f32)
        # shared junk outputs for accum passes — one per engine, so the
        # WAW chains stay engine-internal (in-order: zero cost) and never
        # serialize ACT against DVE
        ajunk = cpool.tile([P, F], _dt)
        djunk = cpool.tile([P, F], _dt)
        # serial-mode rep-serialization tokens (see image_body)
        tokD = cpool.tile([P, 1], _f32)
        tokA = cpool.tile([P, 1], _f32)
        tokP = cpool.tile([8, 1], _f32)
        tokX = cpool.tile([8, 1], _f32)
        sels = []
        for k in range(7):
            sel = cpool.tile([P, 8], _dt, name=f"sel{k}")
            nc.vector.memset(sel[:], 0.0)
            nc.vector.memset(sel[:, k:k + 1], 1.0)
            sels.append(sel)

        def colsum(row, src, reg, first, last):
            """PE selector-matmul column sums of src [128, 3584] into
            cpsum[row, reg:reg+512] (7 uniform 512-wide chunks)."""
            for ci in range(7):
                nc.tensor.matmul(
                    cpsum[:, reg:reg + 512], lhsT=sels[row][:],
                    rhs=src[:, ci * 512:(ci + 1) * 512],
                    start=(first and ci == 0), stop=(last and ci == 6),
                )

        prod_counter = {"apr": 0, "gmp": 0}

        def prod_tile(pool, tag, name):
            i = prod_counter[tag]
            prod_counter[tag] = i + 1
            t = pool.tile([P, FP], _dt, tag=tag, name=name)
            if i < 3:
                # zero this physical buffer's pad columns once; products
                # only ever write cols [0:F], so the pad stays zero
                nc.vector.memset(t[:, F:FP], 0.0)
            return t

        # reps<=1 (and serial=True, the single-shot timing build — an
        # all-engine barrier between reps drains the pipeline so each rep
        # pays full latency) uses the latency-optimized variant: GPSIMD
        # carries one fewer serial product, c_3 sums on DVE, and the PSUM
        # groups close at channel 2 so the endgame never waits on the
        # GPSIMD chain + a cold-PE column sum.
        latency_mode = reps <= 1 or serial

        def image_body(rep, last_rep):
            # serial mode: PSUM groups open/close every rep, and rep r+1's
            # leading DMA is data-gated on tokens that depend on rep r's
            # final accumulator state on every engine — the DMA queue is
            # FIFO, so one gate serializes the whole rep (pipeline drained,
            # each rep pays full single-shot latency).
            fr = rep == 0 or serial    # opens the PSUM groups
            lr = last_rep or serial    # closes the PSUM groups
            Fh = F // 2
            heads = {}
            deferred_sq = []

            # Channel "6" is the text channel; its logits gate everything,
            # so its head runs first. xm is computed IN-PLACE on the x tile
            # (out == in0 on a streaming engine is safe) to save SBUF.
            def head(k):
                xt = xpool.tile([P, F], _dt, tag="xin", name=f"xk{k}_r{rep}")
                gt = gpool.tile([P, F], _dt, tag="gin", name=f"gk{k}_r{rep}")
                if k == 6:
                    m_t = mkpool.tile([P, F], _dt, tag="m_t",
                                      name=f"m_t_r{rep}")
                    if serial and rep > 0:
                        # gate: junk-write into the DMA target, reading the
                        # previous rep's tokens (values never used)
                        nc.vector.tensor_tensor(
                            tokX[:], tokD[0:8, 0:1], tokP[:], _ALU.add)
                        nc.vector.tensor_tensor(
                            tokX[:], tokX[:], tokA[0:8, 0:1], _ALU.add)
                        nc.vector.tensor_tensor(
                            m_t[0:8, 0:1], tokX[:], tokX[:], _ALU.mult)
                    nc.sync.dma_start(m_t[:], _img_ap(m_d.ap()))
                    nc.sync.dma_start(xt[:], _img_ap(x_d.ap()[k]))
                    nc.sync.dma_start(gt[:], _img_ap(g_d.ap()[k]))
                    mb_t = mkpool.tile([P, F], _dt, tag="mb_t",
                                       name=f"mb_t_r{rep}", bufs=1)
                    nc.vector.tensor_scalar(
                        mb_t[:], m_t[:], 2.0 * BIG, -BIG, _ALU.mult, _ALU.add
                    )
                    nc.vector.tensor_tensor(xt[:], xt[:], mb_t[:], _ALU.min)
                    sbt = sbpool.tile([P, F], _dt, tag="sbp",
                                      name=f"sb{k}_r{rep}")
                    nc.scalar.activation(sbt[:], xt[:], _AF.Sigmoid)
                    # xm6 > 0  <=>  (x6 > 0) and m: M comes from one is_gt
                    M_t = mkpool.tile([P, F], _dt, tag="M_t",
                                      name=f"M_t_r{rep}")
                    Mb_t = mkpool.tile([P, F], _dt, tag="Mb_t",
                                       name=f"Mb_t_r{rep}")
                    nc.vector.tensor_scalar(M_t[:], xt[:], 0.0, 0.0,
                                            _ALU.is_gt, _ALU.add)
                    nc.vector.tensor_scalar(
                        Mb_t[:], M_t[:], 2.0 * BIG, -BIG, _ALU.mult, _ALU.add
                    )
                    heads["masks"] = (m_t, M_t, Mb_t)
                    # c_t product (needs only g6, m — not the sigmoid);
                    # it is ready earliest, so it goes on slow GPSIMD
                    cp = prod_tile(gmpool, "gmp", f"cp{k}_r{rep}")
                    nc.gpsimd.tensor_mul(cp[:, :F], gt[:], m_t[:])
                    colsum(6, cp, _RC, fr, False)
                    heads[k] = (xt, gt, sbt, None)
                elif k < 5:
                    m_t, M_t, Mb_t = heads["masks"]
                    nc.sync.dma_start(xt[:], _img_ap(x_d.ap()[k]))
                    if k < 4:
                        nc.sync.dma_start(gt[:], _img_ap(g_d.ap()[k]))
                    nc.vector.tensor_tensor(xt[:], xt[:], Mb_t[:], _ALU.min)
                    sbt = sbpool.tile([P, F], _dt, tag="sbp",
                                      name=f"sb{k}_r{rep}")
                    nc.scalar.activation(sbt[:], xt[:], _AF.Sigmoid)
                    cp = None
                    if k < (3 if latency_mode else 4):
                        # c-product on GPSIMD (runs ahead: needs only g, M)
                        cp = prod_tile(gmpool, "gmp", f"cp{k}_r{rep}")
                        nc.gpsimd.tensor_mul(cp[:, :F], gt[:], M_t[:])
                    heads[k] = (xt, gt, sbt, cp)
                else:
                    # last channel split in two half-tiles so the
                    # post-final-DMA compute tail is short (single-shot)
                    m_t, M_t, Mb_t = heads["masks"]
                    sbts = []
                    for h in range(2):
                        xs = xt[:, h * Fh:(h + 1) * Fh]
                        src = _img_ap(x_d.ap()[k])
                        nc.sync.dma_start(xs, src[:, h * Fh:(h + 1) * Fh])
                        nc.vector.tensor_tensor(
                            xs, xs, Mb_t[:, h * Fh:(h + 1) * Fh], _ALU.min
                        )
                        sbt = sbpool.tile([P, F], _dt, tag="sbp",
                                          name=f"sbL{h}_r{rep}")
                        nc.scalar.activation(sbt[:, :Fh], xs, _AF.Sigmoid)
                        sbts.append(sbt)
                    heads[k] = (xt, gt, sbts, None)

            def tail(k):
                m_t, M_t, Mb_t = heads["masks"]
                if k == 6:
                    xt, gt, sbt, _ = heads[k]
                    # b_t = sum sb6^2 (ACT fused accum; junk out -> ajunk)
                    nc.scalar.activation(
                        ajunk[:], sbt[:], _AF.Square,
                        accum_out=acc_act[:, 0:1],
                    )
                    ap = prod_tile(apool, "apr", f"ap{k}_r{rep}")
                    nc.vector.tensor_tensor(ap[:, :F], gt[:], sbt[:],
                                            _ALU.mult)
                    colsum(6, ap, _RA, fr, False)
                elif k < 5:
                    xt, gt, sbt, cp = heads[k]
                    if k == 4:
                        # g4 rides the DMA queue after the x5 halves, so
                        # the sigmoid chain is never starved of logits
                        nc.sync.dma_start(gt[:], _img_ap(g_d.ap()[k]))
                    if k < 3:
                        # b: ACT Square with fused accum
                        nc.scalar.activation(
                            ajunk[:], sbt[:], _AF.Square,
                            accum_out=acc_act[:, 1 + k:2 + k],
                        )
                    else:
                        # defer sq3/sq4 to the end of the ACT queue so the
                        # last sigmoids (ch 5 halves) run as early as
                        # possible in the single-shot case
                        deferred_sq.append((k, sbt))
                    # a-product on DVE; sum on PE for k<4, TS-accum for
                    # k=4 (late-landing sums stay off the cold-PE path)
                    ap = prod_tile(apool, "apr", f"ap{k}_r{rep}")
                    nc.vector.tensor_tensor(ap[:, :F], gt[:], sbt[:],
                                            _ALU.mult)
                    kstop = 2 if latency_mode else 3
                    if k < 4:
                        colsum(k, ap, _RA, False, k == kstop and lr)
                    else:
                        nc.vector.tensor_scalar(
                            djunk[:], ap[:, :F], 0.0, 0.0, _ALU.add, _ALU.add,
                            accum_out=acc_dve[:, 0:1],
                        )
                    if k == 3 and latency_mode:
                        cp = prod_tile(gmpool, "gmp", f"cp{k}_r{rep}")
                        nc.vector.tensor_tensor(cp[:, :F], gt[:], M_t[:],
                                                _ALU.mult)
                        nc.vector.tensor_scalar(
                            djunk[:], cp[:, :F], 0.0, 0.0, _ALU.add, _ALU.add,
                            accum_out=acc_dve[:, 3:4],
                        )
                    elif k < 4:
                        colsum(k, cp, _RC, False, k == kstop and lr)
                    else:
                        cp = prod_tile(gmpool, "gmp", f"cp{k}_r{rep}")
                        nc.vector.tensor_tensor(cp[:, :F], gt[:], M_t[:],
                                                _ALU.mult)
                        nc.vector.tensor_scalar(
                            djunk[:], cp[:, :F], 0.0, 0.0, _ALU.add, _ALU.add,
                            accum_out=acc_dve[:, 1:2],
                        )
                else:
                    # channel 5: everything sums straight into acc_dve via
                    # TS copy-accums — short post-final-DMA tail, no PE.
                    xt, gt, sbts, _ = heads[k]
                    srcg = _img_ap(g_d.ap()[k])
                    for h in range(2):
                        nc.sync.dma_start(gt[:, h * Fh:(h + 1) * Fh],
                                          srcg[:, h * Fh:(h + 1) * Fh])
                    cp = prod_tile(gmpool, "gmp", f"cp{k}_r{rep}")
                    nc.vector.tensor_tensor(cp[:, :F], gt[:], M_t[:],
                                            _ALU.mult)
                    nc.vector.tensor_scalar(
                        djunk[:], cp[:, :F], 0.0, 0.0, _ALU.add, _ALU.add,
                        accum_out=acc_dve[:, 12:13],
                    )
                    ap = prod_tile(apool, "apr", f"ap{k}_r{rep}")
                    for h in range(2):
                        sbt = sbts[h]
                        xs = xt[:, h * Fh:(h + 1) * Fh]
                        gs = gt[:, h * Fh:(h + 1) * Fh]
                        # b_5 half: TT square into sbt's free half, then
                        # TS copy-accum (junk out -> the spent x half)
                        nc.vector.tensor_tensor(
                            sbt[:, Fh:2 * Fh], sbt[:, :Fh], sbt[:, :Fh],
                            _ALU.mult
                        )
                        nc.vector.tensor_scalar(
                            xs, sbt[:, Fh:2 * Fh], 0.0, 0.0, _ALU.add, _ALU.add,
                            accum_out=acc_dve[:, 10 + h:11 + h],
                        )
                        # a_5 half: TT product + TS copy-accum
                        nc.vector.tensor_tensor(
                            ap[:, h * Fh:(h + 1) * Fh], gs, sbt[:, :Fh],
                            _ALU.mult
                        )
                        nc.vector.tensor_scalar(
                            xs, ap[:, h * Fh:(h + 1) * Fh], 0.0, 0.0,
                            _ALU.add, _ALU.add,
                            accum_out=acc_dve[:, 6 + h:7 + h],
                        )

            # software pipelining, skew 2: tail(k) is emitted two channel
            # heads later, so each engine's in-order queue never blocks on
            # a cross-engine dependency that hasn't had time to finish.
            # In latency mode tail(3) precedes tail(2) so the group-closing
            # channel-2 column sums are the very last PE work.
            order = [6, 0, 1, 2, 3, 4, 5]
            tails = [6, 0, 1, 3, 2, 4, 5] if latency_mode else order
            ti = 0
            for i, k in enumerate(order):
                head(k)
                if i >= 2:
                    tail(tails[ti])
                    ti += 1
            while ti < len(tails):
                tail(tails[ti])
                ti += 1
            for k, sbt in deferred_sq:
                nc.scalar.activation(
                    ajunk[:], sbt[:], _AF.Square,
                    accum_out=acc_act[:, 1 + k:2 + k],
                )

        for rep in range(reps):
            image_body(rep, rep == reps - 1)
            if serial and rep < reps - 1:
                # tokens covering each engine's rep-r work:
                # DVE accums -> tokD; ACT squares -> tokA; PE/GPSIMD column
                # sums (groups closed this rep) -> tokP
                nc.vector.tensor_scalar(tokD[:], acc_dve[:, 0:1], 0.0, 0.0,
                                        _ALU.add, _ALU.add)
                nc.scalar.activation(tokA[:], acc_act[:, 0:1], _AF.Copy)
                nc.vector.tensor_scalar(tokP[:], cpsum[:, 0:1], 0.0, 0.0,
                                        _ALU.add, _ALU.add)

        # cross-partition reduction of the accum columns with one
        # ones-vector matmul per accumulator: res row i <- sum_p acc[p, i]
        pr = ppool.tile([16, 2], _f32, tag="pr")
        nc.tensor.matmul(pr[:, 0:1], lhsT=acc_dve[:], rhs=ones_f[:],
                         start=True, stop=True)
        nc.tensor.matmul(pr[:, 1:2], lhsT=acc_act[:], rhs=ones_f[:],
                         start=True, stop=True)
        res_sb = cpool.tile([16, 2], _f32)
        nc.scalar.copy(res_sb[:], pr[:])
        nc.sync.dma_start(res_d.ap(), res_sb[:])
        # collapse the two PSUM regions to [8,2] (free-dim sums);
        # with reps=0 nothing ever wrote cpsum, so skip the reads
        if reps > 0:
            for j, reg in enumerate((_RC, _RA)):
                nc.vector.tensor_scalar(
                    rjunk[:], cpsum[:, reg:reg + 512], 0.0, 0.0,
                    _ALU.add, _ALU.add,
                    accum_out=res2_sb[:, j:j + 1],
                )
        nc.sync.dma_start(res2_d.ap(), res2_sb[:])

    nc.compile()
    return nc


_CACHE = {}


def _get_nc():
    if "nc" not in _CACHE:
        _CACHE["nc"] = build_nc(debug=False)
    return _CACHE["nc"]


def _combine(res_list, res2_list):
    """Per-image [16,2] + [8,2] device sums -> (loss_text, loss_kernels, loss).

    res2 col 0 = c sums (PE c-region), col 1 = a sums (PE a-region); rows
    0..4 = kernel channels 0..4, row 6 = text. acc_dve (res col 0): cols
    6,7 = a_5 halves, 9 = b_4, 10,11 = b_5 halves, 12 = c_5. acc_act (res
    col 1): col 0 = b_t, cols 1..4 = b_0..b_3.
    Returns None if the OHEM fast-path precondition fails for any image.
    """
    lt_b = np.zeros(B, np.float64)
    lk_b = np.zeros(B, np.float64)
    for b in range(B):
        v = np.asarray(res_list[b], np.float64)
        w = np.asarray(res2_list[b], np.float64)
        a_t = w[6, 1]
        b_t = v[0, 1]
        c_t = w[6, 0]
        pos_num = c_t                    # sum(gt_text * m), exact integer
        # sel == m iff pos_num == 0 (fallback) or neg_num == total_neg,
        # i.e. RATIO*pos_num >= total_neg = N - sum_g. Since sum_g >=
        # sum_g*m = pos_num, (RATIO+1)*pos_num >= N is sufficient.
        if not (pos_num == 0 or (RATIO + 1) * pos_num >= N):
            return None
        lt_b[b] = 1.0 - 2.0 * a_t / (b_t + 0.001 + c_t + 0.001)
        lk = 0.0
        for k in range(NK):
            a_k = w[k, 1] if k < 4 else (
                v[0, 0] if k == 4 else v[6, 0] + v[7, 0])
            if k < 3:
                c_k = w[k, 0]
            elif k == 3:
                c_k = v[3, 0]     # latency-mode NEFF: c_3 via acc_dve
            else:
                c_k = v[1, 0] if k == 4 else v[12, 0]
            b_k = v[1 + k, 1] if k < 5 else v[10, 0] + v[11, 0]
            lk += 1.0 - 2.0 * a_k / (b_k + 0.001 + c_k + 0.001)
        lk_b[b] = lk / NK
    lt = np.float32(lt_b.mean())
    lk = np.float32(lk_b.mean())
    loss = np.float32(LAMBDA) * lt + np.float32(1.0 - LAMBDA) * lk
    return (lt, lk, np.float32(loss))


def _numpy_reference(outputs, labels, training_masks):
    """Full-fidelity host fallback (mirrors the original loss exactly)."""
    def sigmoid(z):
        return 1.0 / (1.0 + np.exp(-z, dtype=np.float64))

    texts = outputs[:, -1].reshape(B, N).astype(np.float64)
    kernels = outputs[:, :-1].reshape(B, NK, N).astype(np.float64)
    gt_texts = labels[:, -1].reshape(B, N).astype(np.float64)
    gt_kernels = labels[:, :-1].reshape(B, NK, N).astype(np.float64)
    tm = training_masks.reshape(B, N).astype(np.float64)

    pos = gt_texts > 0.5
    pos_num = np.sum(pos & (tm > 0.5), axis=1)
    neg = ~pos
    total_neg = np.sum(neg, axis=1)
    neg_num = np.minimum(pos_num * RATIO, total_neg)
    neg_scores = np.where(neg, texts, -np.inf)
    sorted_desc = -np.sort(-neg_scores, axis=1)
    idx = np.clip(neg_num - 1, 0, N - 1)
    thr = np.take_along_axis(sorted_desc, idx[:, None], axis=1)
    sel = (((texts >= thr) | pos) & (tm > 0.5)).astype(np.float64)
    fallback = (pos_num == 0) | (neg_num == 0)
    sel = np.where(fallback[:, None], tm, sel)

    def dice(inp, target, mask):
        p = sigmoid(inp) * mask
        t = target * mask
        a = np.sum(p * t, axis=-1)
        bb = np.sum(p * p, axis=-1) + 0.001
        cc = np.sum(t * t, axis=-1) + 0.001
        return 1.0 - 2.0 * a / (bb + cc)

    loss_text = dice(texts, gt_texts, sel).mean()
    sel_k = ((sigmoid(texts) > 0.5) & (tm > 0.5)).astype(np.float64)
    loss_kernels = dice(kernels, gt_kernels, sel_k[:, None, :]).mean(axis=1).mean()
    loss = LAMBDA * loss_text + (1.0 - LAMBDA) * loss_kernels
    return (np.float32(loss_text), np.float32(loss_kernels), np.float32(loss))


def kernel(outputs, labels, training_masks):
    outputs = np.asarray(outputs, dtype=np.float32)
    labels = np.asarray(labels, dtype=np.float32)
    training_masks = np.asarray(training_masks, dtype=np.float32)
    assert outputs.shape == (B, C, H, W)

    bf16 = _np_bf16()
    nc = _get_nc()
    in_maps = [
        {
            "x": np.ascontiguousarray(outputs[b]).astype(bf16),
            "g": np.ascontiguousarray(labels[b]).astype(bf16),
            "m": np.ascontiguousarray(training_masks[b]).astype(bf16),
        }
        for b in range(B)
    ]
    r = None
    for attempt in range(3):
        try:
            r = run_bass_kernel_spmd(
                nc, in_maps, list(range(NCORES)),
                trace=_CACHE.get("trace", False),
            )
            break
        except Exception:
            if attempt == 2:
                raise
            _CACHE.pop("nc", None)
            nc = _get_nc()
    _CACHE["last_result"] = r
    res_list = [r.results[b]["res"] for b in range(B)]
    res2_list = [r.results[b]["res2"] for b in range(B)]
    out = _combine(res_list, res2_list)
    if out is None:
        # OHEM threshold is not the minimum negative score -> exact host path
        out = _numpy_reference(outputs, labels, training_masks)
    return out


# revision 28
# speedup vs baseline: 13.2644x; 1.8738x over previous
"""PSENet-style OHEM + dice loss on 8 Trainium2 NeuronCores — bf16 edition.

Data-parallel over the batch: core b processes image b entirely on-chip.
All inputs are cast to bf16 on the host (labels/masks are exactly 0/1 so
they are lossless; logits lose ~0.4% relative, far below the fp32 noise
floor of the 400k-element dice sums). This halves HBM traffic per image
from 24.5 MB to 12.25 MB — the kernel is HBM-bound at ~358 GB/s/core, so
the DMA stream (~34 us/image) sets the steady-state floor.

HW-measured per-pass costs at [128, 3200] bf16 (micro-benched on these
cores with bench.py — the CoreSim cost model is badly wrong for several):
  DVE tensor_tensor ~0.75us, tensor_scalar ~0.3us,
  scalar_tensor_tensor+accum ~1.75us (fused product+sum, 2x mode),
  ACT sigmoid ~1.1us, Square+accum ~1.5us, GPSIMD mul ~3.2us,
  tensor_scalar+accum ~2.7us (the reduce variant drops to 1x - avoided).

Schedule per kernel channel k (mask M = (x_text>0)*m, Mb = (2M-1)*BIG):
    xm = min(x, Mb)      DVE TT, in-place on the x tile
                         (sig(xm) = sig(x)*M up to sig(-BIG) ~ 2e-22)
    sb = sigmoid(xm)     ACT
    b  = sum sb^2        ACT Square + accum_out     (fp32 accumulators)
    a  = sum g*sb        DVE STT + accum_out
    c  = sum g*M         DVE STT + accum_out
Text channel: same with mask m (mb = (2m-1)*BIG); M = is_gt(xm6, 0) in a
single TS pass since xm6 > 0 <=> (x6>0 and m). pos_num = c_t feeds the
host-verified OHEM fast path. The last channel is processed as two
half-tiles so the post-final-DMA compute tail is halved. Channel heads
(DMA + xm + sigmoid) are emitted one channel ahead of tails (b/a/c sums)
so DVE's in-order queue never stalls on ACT.

Totals per image: DVE ~31us, ACT ~19us, DMA ~34us -> DMA-bound. accum
columns are cross-partition reduced by one ones-matmul per accumulator at
the end; the host combines 8 x 16x2 floats into the three scalars.

OHEM: for these inputs 3*pos_num >= total_neg for every image, so the
selected mask is exactly the training mask. The host VERIFIES
(RATIO+1)*pos_num >= N (pos_num = c_t, exact: 0/1 values, fp32 accum)
and falls back to a full host reference if it ever fails.

build_nc(reps, serial=True) emits a timing variant whose reps are
data-gated on the previous rep's accumulators (pipeline drained), used by
test.py to measure single-image latency with a large-signal difference.
"""

import os
import sys

import numpy as np

for _p in ("/opt/trn_rl_repo", "/root/.axon_site/_ro/trn_rl_repo"):
    if os.path.isdir(_p) and _p not in sys.path:
        sys.path.append(_p)

import concourse.bacc as bacc
import concourse.tile as tile
from concourse import mybir
from concourse.bass_utils import run_bass_kernel_spmd

B, C, H, W = 8, 7, 640, 640
NK = C - 1            # kernel channels
N = H * W             # pixels per image
P = 128               # SBUF partitions
F = N // P            # free dim per tile (3200)
BIG = 50.0
NCORES = 8
LAMBDA = 0.7
RATIO = 3

_dt = mybir.dt.bfloat16
_f32 = mybir.dt.float32
_AF = mybir.ActivationFunctionType
_ALU = mybir.AluOpType

_NP_BF16 = None


def _np_bf16():
    global _NP_BF16
    if _NP_BF16 is None:
        _NP_BF16 = mybir.dt.np(mybir.dt.bfloat16)
    return _NP_BF16


def _img_ap(dram_ap):
    """[H, W] dram slab -> [128, 3200] partition-major access pattern."""
    return dram_ap.rearrange("(p q) w -> p (q w)", p=P)


def build_nc(debug=False, reps=1, serial=False):
    nc = bacc.Bacc("TRN2", target_bir_lowering=False, debug=debug)
    x_d = nc.dram_tensor("x", [C, H, W], _dt, kind="ExternalInput")
    g_d = nc.dram_tensor("g", [C, H, W], _dt, kind="ExternalInput")
    m_d = nc.dram_tensor("m", [H, W], _dt, kind="ExternalInput")
    res_d = nc.dram_tensor("res", [16, 2], _f32, kind="ExternalOutput")

    with (
        tile.TileContext(nc) as tc,
        tc.tile_pool(name="const", bufs=1) as cpool,
        tc.tile_pool(name="mask", bufs=2) as mkpool,
        tc.tile_pool(name="xin", bufs=5) as xpool,
        tc.tile_pool(name="gin", bufs=5) as gpool,
        tc.tile_pool(name="sbp", bufs=4) as sbpool,
        tc.tile_pool(name="ps", bufs=1, space="PSUM") as ppool,
    ):
        # fp32 per-partition accumulators (fused accum_out targets).
        # acc_dve (DVE STT): 0=a_t, 1..5=a_k(k=0..4), 6,7=a_5 halves,
        #                    8=c_t(=pos_num), 9..13=c_k(k=0..4), 14=c_5
        # acc_act (ACT Square): 0=b_t, 1..5=b_k(k=0..4), 6,7=b_5 halves
        acc_dve = cpool.tile([P, 16], _f32)
        acc_act = cpool.tile([P, 16], _f32)
        ones_f = cpool.tile([P, 1], _f32)
        nc.gpsimd.memset(ones_f[:], 1.0)
        nc.vector.memset(acc_dve[:], 0.0)
        nc.scalar.memzero(acc_act[:])
        # shared junk outputs for accum passes — one per engine, so the
        # WAW chains stay engine-internal (in-order: zero cost)
        ajunk = cpool.tile([P, F], _dt)
        djunk = cpool.tile([P, F], _dt)
        # serial-mode rep-serialization tokens (see below)
        tokD = cpool.tile([P, 1], _f32)
        tokA = cpool.tile([P, 1], _f32)
        tokX = cpool.tile([P, 1], _f32)

        def image_body(rep):
            Fh = F // 2
            heads = {}

            def head(k):
                xt = xpool.tile([P, F], _dt, tag="xin", name=f"xk{k}_r{rep}")
                gt = gpool.tile([P, F], _dt, tag="gin", name=f"gk{k}_r{rep}")
                if k == 6:
                    # text channel first: its logits gate everything
                    m_t = mkpool.tile([P, F], _dt, tag="m_t",
                                      name=f"m_t_r{rep}")
                    if serial and rep > 0:
                        # gate: junk-write into the DMA target, reading the
                        # previous rep's tokens; the DMA queue is FIFO so
                        # this serializes the whole rep (values unused)
                        nc.vector.tensor_tensor(
                            tokX[:], tokD[:], tokA[:], _ALU.add)
                        nc.vector.tensor_tensor(
                            m_t[:, 0:1], tokX[:], tokX[:], _ALU.mult)
                    nc.sync.dma_start(m_t[:], _img_ap(m_d.ap()))
                    nc.sync.dma_start(xt[:], _img_ap(x_d.ap()[k]))
                    nc.sync.dma_start(gt[:], _img_ap(g_d.ap()[k]))
                    mb_t = mkpool.tile([P, F], _dt, tag="mb_t",
                                       name=f"mb_t_r{rep}", bufs=1)
                    nc.vector.tensor_scalar(
                        mb_t[:], m_t[:], 2.0 * BIG, -BIG, _ALU.mult,
                        _ALU.add
                    )
                    nc.vector.tensor_tensor(xt[:], xt[:], mb_t[:], _ALU.min)
                    sbt = sbpool.tile([P, F], _dt, tag="sbp",
                                      name=f"sb{k}_r{rep}")
                    nc.scalar.activation(sbt[:], xt[:], _AF.Sigmoid)
                    # xm6 > 0  <=>  (x6 > 0) and m: M from one is_gt pass
                    M_t = mkpool.tile([P, F], _dt, tag="M_t",
                                      name=f"M_t_r{rep}")
                    Mb_t = mkpool.tile([P, F], _dt, tag="Mb_t",
                                       name=f"Mb_t_r{rep}")
                    nc.vector.tensor_scalar(M_t[:], xt[:], 0.0, 0.0,
                                            _ALU.is_gt, _ALU.add)
                    nc.vector.tensor_scalar(
                        Mb_t[:], M_t[:], 2.0 * BIG, -BIG, _ALU.mult,
                        _ALU.add
                    )
                    heads["masks"] = (m_t, M_t, Mb_t)
                    heads[k] = (xt, gt, sbt)
                elif k < 5:
                    m_t, M_t, Mb_t = heads["masks"]
                    nc.sync.dma_start(xt[:], _img_ap(x_d.ap()[k]))
                    nc.sync.dma_start(gt[:], _img_ap(g_d.ap()[k]))
                    nc.vector.tensor_tensor(xt[:], xt[:], Mb_t[:], _ALU.min)
                    sbt = sbpool.tile([P, F], _dt, tag="sbp",
                                      name=f"sb{k}_r{rep}")
                    nc.scalar.activation(sbt[:], xt[:], _AF.Sigmoid)
                    heads[k] = (xt, gt, sbt)
                else:
                    # last channel in two half-tiles: short post-final-DMA
                    # tail (single-shot latency)
                    m_t, M_t, Mb_t = heads["masks"]
                    sbts = []
                    for h in range(2):
                        xs = xt[:, h * Fh:(h + 1) * Fh]
                        src = _img_ap(x_d.ap()[k])
                        nc.sync.dma_start(xs, src[:, h * Fh:(h + 1) * Fh])
                        srcg = _img_ap(g_d.ap()[k])
                        nc.sync.dma_start(gt[:, h * Fh:(h + 1) * Fh],
                                          srcg[:, h * Fh:(h + 1) * Fh])
                        nc.vector.tensor_tensor(
                            xs, xs, Mb_t[:, h * Fh:(h + 1) * Fh], _ALU.min
                        )
                        sbt = sbpool.tile([P, F], _dt, tag="sbp",
                                          name=f"sbL{h}_r{rep}")
                        nc.scalar.activation(sbt[:, :Fh], xs, _AF.Sigmoid)
                        sbts.append(sbt)
                    heads[k] = (xt, gt, sbts)

            def tail(k):
                m_t, M_t, Mb_t = heads["masks"]
                if k == 6:
                    xt, gt, sbt = heads[k]
                    nc.scalar.activation(
                        ajunk[:], sbt[:], _AF.Square,
                        accum_out=acc_act[:, 0:1],
                    )
                    nc.vector.scalar_tensor_tensor(
                        djunk[:], gt[:], 1.0, sbt[:], _ALU.mult, _ALU.mult,
                        accum_out=acc_dve[:, 0:1],
                    )
                    nc.vector.scalar_tensor_tensor(
                        djunk[:], gt[:], 1.0, m_t[:], _ALU.mult, _ALU.mult,
                        accum_out=acc_dve[:, 8:9],
                    )
                elif k < 5:
                    xt, gt, sbt = heads[k]
                    nc.scalar.activation(
                        ajunk[:], sbt[:], _AF.Square,
                        accum_out=acc_act[:, 1 + k:2 + k],
                    )
                    nc.vector.scalar_tensor_tensor(
                        djunk[:], gt[:], 1.0, sbt[:], _ALU.mult, _ALU.mult,
                        accum_out=acc_dve[:, 1 + k:2 + k],
                    )
                    nc.vector.scalar_tensor_tensor(
                        djunk[:], gt[:], 1.0, M_t[:], _ALU.mult, _ALU.mult,
                        accum_out=acc_dve[:, 9 + k:10 + k],
                    )
                else:
                    xt, gt, sbts = heads[k]
                    for h in range(2):
                        sbt = sbts[h]
                        gs = gt[:, h * Fh:(h + 1) * Fh]
                        nc.scalar.activation(
                            ajunk[:, :Fh], sbt[:, :Fh], _AF.Square,
                            accum_out=acc_act[:, 6 + h:7 + h],
                        )
                        nc.vector.scalar_tensor_tensor(
                            djunk[:, :Fh], gs, 1.0, sbt[:, :Fh],
                            _ALU.mult, _ALU.mult,
                            accum_out=acc_dve[:, 6 + h:7 + h],
                        )
                    # c_5 on the full tile (both g halves have landed)
                    nc.vector.scalar_tensor_tensor(
                        djunk[:], gt[:], 1.0, M_t[:], _ALU.mult, _ALU.mult,
                        accum_out=acc_dve[:, 14:15],
                    )

            # software pipelining, skew 1 (ACT sigmoids are ~1.1us, so one
            # channel of lookahead keeps DVE's in-order queue fed)
            order = [6, 0, 1, 2, 3, 4, 5]
            pend = []
            for k in order:
                head(k)
                if pend:
                    tail(pend.pop(0))
                pend.append(k)
            for k in pend:
                tail(k)

        for rep in range(reps):
            image_body(rep)
            if serial and rep < reps - 1:
                # tokens covering each engine's rep work: DVE accums ->
                # tokD; ACT squares -> tokA (both engines in-order)
                nc.vector.tensor_scalar(tokD[:], acc_dve[:, 0:1], 1.0, 0.0,
                                        _ALU.mult, _ALU.add)
                nc.scalar.activation(tokA[:], acc_act[:, 0:1], _AF.Copy)

        # cross-partition reduction of the accum columns with one
        # ones-vector matmul per accumulator: res row i <- sum_p acc[p, i]
        pr = ppool.tile([16, 2], _f32, tag="pr")
        nc.tensor.matmul(pr[:, 0:1], lhsT=acc_dve[:], rhs=ones_f[:],
                         start=True, stop=True)
        nc.tensor.matmul(pr[:, 1:2], lhsT=acc_act[:], rhs=ones_f[:],
                         start=True, stop=True)
        res_sb = cpool.tile([16, 2], _f32)
        nc.scalar.copy(res_sb[:], pr[:])
        nc.sync.dma_start(res_d.ap(), res_sb[:])

    nc.compile()
    return nc


_CACHE = {}


def _get_nc():
    if "nc" not in _CACHE:
        _CACHE["nc"] = build_nc(debug=False)
    return _CACHE["nc"]


def _combine(res_list):
    """Per-image [16,2] device sums -> (loss_text, loss_kernels, loss).

    res col 0 = acc_dve (a and c sums), col 1 = acc_act (b sums); see
    build_nc for the column layout.
    Returns None if the OHEM fast-path precondition fails for any image.
    """
    lt_b = np.zeros(B, np.float64)
    lk_b = np.zeros(B, np.float64)
    for b in range(B):
        v = np.asarray(res_list[b], np.float64)
        a_t, b_t, c_t = v[0, 0], v[0, 1], v[8, 0]
        pos_num = c_t                    # sum(gt_text * m), exact integer
        # sel == m iff pos_num == 0 (fallback) or neg_num == total_neg,
        # i.e. RATIO*pos_num >= total_neg = N - sum_g. Since sum_g >=
        # sum_g*m = pos_num, (RATIO+1)*pos_num >= N is sufficient.
        if not (pos_num == 0 or (RATIO + 1) * pos_num >= N):
            return None
        lt_b[b] = 1.0 - 2.0 * a_t / (b_t + 0.001 + c_t + 0.001)
        lk = 0.0
        for k in range(NK):
            if k < 5:
                a_k, b_k, c_k = v[1 + k, 0], v[1 + k, 1], v[9 + k, 0]
            else:
                a_k = v[6, 0] + v[7, 0]
                b_k = v[6, 1] + v[7, 1]
                c_k = v[14, 0]
            lk += 1.0 - 2.0 * a_k / (b_k + 0.001 + c_k + 0.001)
        lk_b[b] = lk / NK
    lt = np.float32(lt_b.mean())
    lk = np.float32(lk_b.mean())
    loss = np.float32(LAMBDA) * lt + np.float32(1.0 - LAMBDA) * lk
    return (lt, lk, np.float32(loss))


def _numpy_reference(outputs, labels, training_masks):
    """Full-fidelity host fallback (mirrors the original loss exactly)."""
    def sigmoid(z):
        return 1.0 / (1.0 + np.exp(-z, dtype=np.float64))

    texts = outputs[:, -1].reshape(B, N).astype(np.float64)
    kernels = outputs[:, :-1].reshape(B, NK, N).astype(np.float64)
    gt_texts = labels[:, -1].reshape(B, N).astype(np.float64)
    gt_kernels = labels[:, :-1].reshape(B, NK, N).astype(np.float64)
    tm = training_masks.reshape(B, N).astype(np.float64)

    pos = gt_texts > 0.5
    pos_num = np.sum(pos & (tm > 0.5), axis=1)
    neg = ~pos
    total_neg = np.sum(neg, axis=1)
    neg_num = np.minimum(pos_num * RATIO, total_neg)
    neg_scores = np.where(neg, texts, -np.inf)
    sorted_desc = -np.sort(-neg_scores, axis=1)
    idx = np.clip(neg_num - 1, 0, N - 1)
    thr = np.take_along_axis(sorted_desc, idx[:, None], axis=1)
    sel = (((texts >= thr) | pos) & (tm > 0.5)).astype(np.float64)
    fallback = (pos_num == 0) | (neg_num == 0)
    sel = np.where(fallback[:, None], tm, sel)

    def dice(inp, target, mask):
        p = sigmoid(inp) * mask
        t = target * mask
        a = np.sum(p * t, axis=-1)
        bb = np.sum(p * p, axis=-1) + 0.001
        cc = np.sum(t * t, axis=-1) + 0.001
        return 1.0 - 2.0 * a / (bb + cc)

    loss_text = dice(texts, gt_texts, sel).mean()
    sel_k = ((sigmoid(texts) > 0.5) & (tm > 0.5)).astype(np.float64)
    loss_kernels = dice(kernels, gt_kernels, sel_k[:, None, :]).mean(axis=1).mean()
    loss = LAMBDA * loss_text + (1.0 - LAMBDA) * loss_kernels
    return (np.float32(loss_text), np.float32(loss_kernels), np.float32(loss))


def kernel(outputs, labels, training_masks):
    outputs = np.asarray(outputs, dtype=np.float32)
    labels = np.asarray(labels, dtype=np.float32)
    training_masks = np.asarray(training_masks, dtype=np.float32)
    assert outputs.shape == (B, C, H, W)

    bf16 = _np_bf16()
    nc = _get_nc()
    in_maps = [
        {
            "x": np.ascontiguousarray(outputs[b]).astype(bf16),
            "g": np.ascontiguousarray(labels[b]).astype(bf16),
            "m": np.ascontiguousarray(training_masks[b]).astype(bf16),
        }
        for b in range(B)
    ]
    r = None
    for attempt in range(3):
        try:
            r = run_bass_kernel_spmd(
                nc, in_maps, list(range(NCORES)),
                trace=_CACHE.get("trace", False),
            )
            break
        except Exception:
            if attempt == 2:
                raise
            _CACHE.pop("nc", None)
            nc = _get_nc()
    _CACHE["last_result"] = r
    res_list = [r.results[b]["res"] for b in range(B)]
    out = _combine(res_list)
    if out is None:
        # OHEM threshold is not the minimum negative score -> exact host path
        out = _numpy_reference(outputs, labels, training_masks)
    return out


# revision 29
# speedup vs baseline: 14.2922x; 1.0775x over previous
"""PSENet-style OHEM + dice loss on 8 Trainium2 NeuronCores — bf16 edition.

Data-parallel over the batch: core b processes image b entirely on-chip.
All inputs are cast to bf16 on the host (labels/masks are exactly 0/1 so
they are lossless; logits lose ~0.4% relative, far below the fp32 noise
floor of the 400k-element dice sums). This halves HBM traffic per image
from 24.5 MB to 12.25 MB — the kernel is HBM-bound at ~358 GB/s/core, so
the DMA stream (~34 us/image) sets the steady-state floor.

HW-measured per-pass costs at [128, 3200] bf16 (micro-benched on these
cores with bench.py — the CoreSim cost model is badly wrong for several):
  DVE tensor_tensor ~0.75us, tensor_scalar ~0.3us,
  scalar_tensor_tensor+accum ~1.75us (fused product+sum, 2x mode),
  ACT sigmoid ~1.1us, Square+accum ~1.5us, GPSIMD mul ~3.2us,
  tensor_scalar+accum ~2.7us (the reduce variant drops to 1x - avoided).

Schedule per kernel channel k (mask M = (x_text>0)*m, Mb = (2M-1)*BIG):
    xm = min(x, Mb)      DVE TT, in-place on the x tile
                         (sig(xm) = sig(x)*M up to sig(-BIG) ~ 2e-22)
    sb = sigmoid(xm)     ACT
    b  = sum sb^2        ACT Square + accum_out     (fp32 accumulators)
    a  = sum g*sb        DVE STT + accum_out
    c  = sum g*M         DVE STT + accum_out
Text channel: same with mask m (mb = (2m-1)*BIG); M = is_gt(xm6, 0) in a
single TS pass since xm6 > 0 <=> (x6>0 and m). pos_num = c_t feeds the
host-verified OHEM fast path. The last channel is processed as two
half-tiles so the post-final-DMA compute tail is halved. Channel heads
(DMA + xm + sigmoid) are emitted one channel ahead of tails (b/a/c sums)
so DVE's in-order queue never stalls on ACT.

Totals per image: DVE ~31us, ACT ~19us, DMA ~34us -> DMA-bound. accum
columns are cross-partition reduced by one ones-matmul per accumulator at
the end; the host combines 8 x 16x2 floats into the three scalars.
HW-measured: steady-state 35.1 us/image (vs 72.4 us for the fp32
predecessor), rel err 2.3e-06.

OHEM: for these inputs 3*pos_num >= total_neg for every image, so the
selected mask is exactly the training mask. The host VERIFIES
(RATIO+1)*pos_num >= N (pos_num = c_t, exact: 0/1 values, fp32 accum)
and falls back to a full host reference if it ever fails.

build_nc(reps, serial=True) emits a timing variant whose reps are
data-gated on the previous rep's accumulators (pipeline drained), used by
test.py to measure single-image latency with a large-signal difference.
"""

import os
import sys

import numpy as np

for _p in ("/opt/trn_rl_repo", "/root/.axon_site/_ro/trn_rl_repo"):
    if os.path.isdir(_p) and _p not in sys.path:
        sys.path.append(_p)

import concourse.bacc as bacc
import concourse.tile as tile
from concourse import mybir
from concourse.bass_utils import run_bass_kernel_spmd

B, C, H, W = 8, 7, 640, 640
NK = C - 1            # kernel channels
N = H * W             # pixels per image
P = 128               # SBUF partitions
F = N // P            # free dim per tile (3200)
BIG = 50.0
NCORES = 8
LAMBDA = 0.7
RATIO = 3

_dt = mybir.dt.bfloat16
_f32 = mybir.dt.float32
_AF = mybir.ActivationFunctionType
_ALU = mybir.AluOpType

_NP_BF16 = None


def _np_bf16():
    global _NP_BF16
    if _NP_BF16 is None:
        _NP_BF16 = mybir.dt.np(mybir.dt.bfloat16)
    return _NP_BF16


def _img_ap(dram_ap):
    """[H, W] dram slab -> [128, 3200] partition-major access pattern."""
    return dram_ap.rearrange("(p q) w -> p (q w)", p=P)


def build_nc(debug=False, reps=1, serial=False):
    nc = bacc.Bacc("TRN2", target_bir_lowering=False, debug=debug)
    x_d = nc.dram_tensor("x", [C, H, W], _dt, kind="ExternalInput")
    g_d = nc.dram_tensor("g", [C, H, W], _dt, kind="ExternalInput")
    m_d = nc.dram_tensor("m", [H, W], _dt, kind="ExternalInput")
    res_d = nc.dram_tensor("res", [16, 2], _f32, kind="ExternalOutput")

    with (
        tile.TileContext(nc) as tc,
        tc.tile_pool(name="const", bufs=1) as cpool,
        tc.tile_pool(name="mask", bufs=2) as mkpool,
        tc.tile_pool(name="xin", bufs=5) as xpool,
        tc.tile_pool(name="gin", bufs=5) as gpool,
        tc.tile_pool(name="sbp", bufs=4) as sbpool,
        tc.tile_pool(name="ps", bufs=1, space="PSUM") as ppool,
    ):
        # fp32 per-partition accumulators (fused accum_out targets).
        # acc_dve (DVE STT): 0=a_t, 1..5=a_k(k=0..4), 6,7=a_5 halves,
        #                    8=c_t(=pos_num), 9..13=c_k(k=0..4), 14=c_5
        # acc_act (ACT Square): 0=b_t, 1..5=b_k(k=0..4), 6,7=b_5 halves
        acc_dve = cpool.tile([P, 16], _f32)
        acc_act = cpool.tile([P, 16], _f32)
        ones_f = cpool.tile([P, 1], _f32)
        nc.gpsimd.memset(ones_f[:], 1.0)
        nc.vector.memset(acc_dve[:], 0.0)
        nc.scalar.memzero(acc_act[:])
        # shared junk outputs for accum passes — one per engine, so the
        # WAW chains stay engine-internal (in-order: zero cost)
        ajunk = cpool.tile([P, F], _dt)
        djunk = cpool.tile([P, F], _dt)
        # serial-mode rep-serialization tokens (see below)
        tokD = cpool.tile([P, 1], _f32)
        tokA = cpool.tile([P, 1], _f32)
        tokX = cpool.tile([P, 1], _f32)

        def image_body(rep):
            Fh = F // 2
            heads = {}

            def head(k):
                xt = xpool.tile([P, F], _dt, tag="xin", name=f"xk{k}_r{rep}")
                gt = gpool.tile([P, F], _dt, tag="gin", name=f"gk{k}_r{rep}")
                if k == 6:
                    # text channel first: its logits gate everything
                    m_t = mkpool.tile([P, F], _dt, tag="m_t",
                                      name=f"m_t_r{rep}")
                    if serial and rep > 0:
                        # gate: junk-write into the DMA target, reading the
                        # previous rep's tokens; the DMA queue is FIFO so
                        # this serializes the whole rep (values unused)
                        nc.vector.tensor_tensor(
                            tokX[:], tokD[:], tokA[:], _ALU.add)
                        nc.vector.tensor_tensor(
                            m_t[:, 0:1], tokX[:], tokX[:], _ALU.mult)
                    nc.sync.dma_start(m_t[:], _img_ap(m_d.ap()))
                    nc.sync.dma_start(xt[:], _img_ap(x_d.ap()[k]))
                    nc.sync.dma_start(gt[:], _img_ap(g_d.ap()[k]))
                    mb_t = mkpool.tile([P, F], _dt, tag="mb_t",
                                       name=f"mb_t_r{rep}", bufs=1)
                    nc.vector.tensor_scalar(
                        mb_t[:], m_t[:], 2.0 * BIG, -BIG, _ALU.mult,
                        _ALU.add
                    )
                    nc.vector.tensor_tensor(xt[:], xt[:], mb_t[:], _ALU.min)
                    sbt = sbpool.tile([P, F], _dt, tag="sbp",
                                      name=f"sb{k}_r{rep}")
                    nc.scalar.activation(sbt[:], xt[:], _AF.Sigmoid)
                    # xm6 > 0  <=>  (x6 > 0) and m: M from one is_gt pass
                    M_t = mkpool.tile([P, F], _dt, tag="M_t",
                                      name=f"M_t_r{rep}")
                    Mb_t = mkpool.tile([P, F], _dt, tag="Mb_t",
                                       name=f"Mb_t_r{rep}")
                    nc.vector.tensor_scalar(M_t[:], xt[:], 0.0, 0.0,
                                            _ALU.is_gt, _ALU.add)
                    nc.vector.tensor_scalar(
                        Mb_t[:], M_t[:], 2.0 * BIG, -BIG, _ALU.mult,
                        _ALU.add
                    )
                    heads["masks"] = (m_t, M_t, Mb_t)
                    heads[k] = (xt, gt, sbt)
                elif k < 5:
                    m_t, M_t, Mb_t = heads["masks"]
                    nc.sync.dma_start(xt[:], _img_ap(x_d.ap()[k]))
                    nc.sync.dma_start(gt[:], _img_ap(g_d.ap()[k]))
                    nc.vector.tensor_tensor(xt[:], xt[:], Mb_t[:], _ALU.min)
                    sbt = sbpool.tile([P, F], _dt, tag="sbp",
                                      name=f"sb{k}_r{rep}")
                    nc.scalar.activation(sbt[:], xt[:], _AF.Sigmoid)
                    heads[k] = (xt, gt, sbt)
                else:
                    # last channel in two half-tiles: short post-final-DMA
                    # tail (single-shot latency)
                    m_t, M_t, Mb_t = heads["masks"]
                    sbts = []
                    for h in range(2):
                        xs = xt[:, h * Fh:(h + 1) * Fh]
                        src = _img_ap(x_d.ap()[k])
                        nc.sync.dma_start(xs, src[:, h * Fh:(h + 1) * Fh])
                        srcg = _img_ap(g_d.ap()[k])
                        nc.sync.dma_start(gt[:, h * Fh:(h + 1) * Fh],
                                          srcg[:, h * Fh:(h + 1) * Fh])
                        nc.vector.tensor_tensor(
                            xs, xs, Mb_t[:, h * Fh:(h + 1) * Fh], _ALU.min
                        )
                        sbt = sbpool.tile([P, F], _dt, tag="sbp",
                                          name=f"sbL{h}_r{rep}")
                        nc.scalar.activation(sbt[:, :Fh], xs, _AF.Sigmoid)
                        sbts.append(sbt)
                    heads[k] = (xt, gt, sbts)

            def tail(k):
                m_t, M_t, Mb_t = heads["masks"]
                if k == 6:
                    xt, gt, sbt = heads[k]
                    nc.scalar.activation(
                        ajunk[:], sbt[:], _AF.Square,
                        accum_out=acc_act[:, 0:1],
                    )
                    nc.vector.scalar_tensor_tensor(
                        djunk[:], gt[:], 1.0, sbt[:], _ALU.mult, _ALU.mult,
                        accum_out=acc_dve[:, 0:1],
                    )
                    nc.vector.scalar_tensor_tensor(
                        djunk[:], gt[:], 1.0, m_t[:], _ALU.mult, _ALU.mult,
                        accum_out=acc_dve[:, 8:9],
                    )
                elif k < 5:
                    xt, gt, sbt = heads[k]
                    nc.scalar.activation(
                        ajunk[:], sbt[:], _AF.Square,
                        accum_out=acc_act[:, 1 + k:2 + k],
                    )
                    nc.vector.scalar_tensor_tensor(
                        djunk[:], gt[:], 1.0, sbt[:], _ALU.mult, _ALU.mult,
                        accum_out=acc_dve[:, 1 + k:2 + k],
                    )
                    nc.vector.scalar_tensor_tensor(
                        djunk[:], gt[:], 1.0, M_t[:], _ALU.mult, _ALU.mult,
                        accum_out=acc_dve[:, 9 + k:10 + k],
                    )
                else:
                    xt, gt, sbts = heads[k]
                    for h in range(2):
                        sbt = sbts[h]
                        gs = gt[:, h * Fh:(h + 1) * Fh]
                        nc.scalar.activation(
                            ajunk[:, :Fh], sbt[:, :Fh], _AF.Square,
                            accum_out=acc_act[:, 6 + h:7 + h],
                        )
                        nc.vector.scalar_tensor_tensor(
                            djunk[:, :Fh], gs, 1.0, sbt[:, :Fh],
                            _ALU.mult, _ALU.mult,
                            accum_out=acc_dve[:, 6 + h:7 + h],
                        )
                    # c_5 on the full tile (both g halves have landed)
                    nc.vector.scalar_tensor_tensor(
                        djunk[:], gt[:], 1.0, M_t[:], _ALU.mult, _ALU.mult,
                        accum_out=acc_dve[:, 14:15],
                    )

            # software pipelining, skew 1 (ACT sigmoids are ~1.1us, so one
            # channel of lookahead keeps DVE's in-order queue fed)
            order = [6, 0, 1, 2, 3, 4, 5]
            pend = []
            for k in order:
                head(k)
                if pend:
                    tail(pend.pop(0))
                pend.append(k)
            for k in pend:
                tail(k)

        for rep in range(reps):
            image_body(rep)
            if serial and rep < reps - 1:
                # tokens covering each engine's rep work: DVE accums ->
                # tokD; ACT squares -> tokA (both engines in-order)
                nc.vector.tensor_scalar(tokD[:], acc_dve[:, 0:1], 1.0, 0.0,
                                        _ALU.mult, _ALU.add)
                nc.scalar.activation(tokA[:], acc_act[:, 0:1], _AF.Copy)

        # cross-partition reduction of the accum columns with one
        # ones-vector matmul per accumulator: res row i <- sum_p acc[p, i]
        pr = ppool.tile([16, 2], _f32, tag="pr")
        nc.tensor.matmul(pr[:, 0:1], lhsT=acc_dve[:], rhs=ones_f[:],
                         start=True, stop=True)
        nc.tensor.matmul(pr[:, 1:2], lhsT=acc_act[:], rhs=ones_f[:],
                         start=True, stop=True)
        res_sb = cpool.tile([16, 2], _f32)
        nc.scalar.copy(res_sb[:], pr[:])
        nc.sync.dma_start(res_d.ap(), res_sb[:])

    nc.compile()
    return nc


_CACHE = {}


def _get_nc():
    if "nc" not in _CACHE:
        _CACHE["nc"] = build_nc(debug=False)
    return _CACHE["nc"]


def _combine(res_list):
    """Per-image [16,2] device sums -> (loss_text, loss_kernels, loss).

    res col 0 = acc_dve (a and c sums), col 1 = acc_act (b sums); see
    build_nc for the column layout.
    Returns None if the OHEM fast-path precondition fails for any image.
    """
    lt_b = np.zeros(B, np.float64)
    lk_b = np.zeros(B, np.float64)
    for b in range(B):
        v = np.asarray(res_list[b], np.float64)
        a_t, b_t, c_t = v[0, 0], v[0, 1], v[8, 0]
        pos_num = c_t                    # sum(gt_text * m), exact integer
        # sel == m iff pos_num == 0 (fallback) or neg_num == total_neg,
        # i.e. RATIO*pos_num >= total_neg = N - sum_g. Since sum_g >=
        # sum_g*m = pos_num, (RATIO+1)*pos_num >= N is sufficient.
        if not (pos_num == 0 or (RATIO + 1) * pos_num >= N):
            return None
        lt_b[b] = 1.0 - 2.0 * a_t / (b_t + 0.001 + c_t + 0.001)
        lk = 0.0
        for k in range(NK):
            if k < 5:
                a_k, b_k, c_k = v[1 + k, 0], v[1 + k, 1], v[9 + k, 0]
            else:
                a_k = v[6, 0] + v[7, 0]
                b_k = v[6, 1] + v[7, 1]
                c_k = v[14, 0]
            lk += 1.0 - 2.0 * a_k / (b_k + 0.001 + c_k + 0.001)
        lk_b[b] = lk / NK
    lt = np.float32(lt_b.mean())
    lk = np.float32(lk_b.mean())
    loss = np.float32(LAMBDA) * lt + np.float32(1.0 - LAMBDA) * lk
    return (lt, lk, np.float32(loss))


def _numpy_reference(outputs, labels, training_masks):
    """Full-fidelity host fallback (mirrors the original loss exactly)."""
    def sigmoid(z):
        return 1.0 / (1.0 + np.exp(-z, dtype=np.float64))

    texts = outputs[:, -1].reshape(B, N).astype(np.float64)
    kernels = outputs[:, :-1].reshape(B, NK, N).astype(np.float64)
    gt_texts = labels[:, -1].reshape(B, N).astype(np.float64)
    gt_kernels = labels[:, :-1].reshape(B, NK, N).astype(np.float64)
    tm = training_masks.reshape(B, N).astype(np.float64)

    pos = gt_texts > 0.5
    pos_num = np.sum(pos & (tm > 0.5), axis=1)
    neg = ~pos
    total_neg = np.sum(neg, axis=1)
    neg_num = np.minimum(pos_num * RATIO, total_neg)
    neg_scores = np.where(neg, texts, -np.inf)
    sorted_desc = -np.sort(-neg_scores, axis=1)
    idx = np.clip(neg_num - 1, 0, N - 1)
    thr = np.take_along_axis(sorted_desc, idx[:, None], axis=1)
    sel = (((texts >= thr) | pos) & (tm > 0.5)).astype(np.float64)
    fallback = (pos_num == 0) | (neg_num == 0)
    sel = np.where(fallback[:, None], tm, sel)

    def dice(inp, target, mask):
        p = sigmoid(inp) * mask
        t = target * mask
        a = np.sum(p * t, axis=-1)
        bb = np.sum(p * p, axis=-1) + 0.001
        cc = np.sum(t * t, axis=-1) + 0.001
        return 1.0 - 2.0 * a / (bb + cc)

    loss_text = dice(texts, gt_texts, sel).mean()
    sel_k = ((sigmoid(texts) > 0.5) & (tm > 0.5)).astype(np.float64)
    loss_kernels = dice(kernels, gt_kernels, sel_k[:, None, :]).mean(axis=1).mean()
    loss = LAMBDA * loss_text + (1.0 - LAMBDA) * loss_kernels
    return (np.float32(loss_text), np.float32(loss_kernels), np.float32(loss))


def kernel(outputs, labels, training_masks):
    outputs = np.asarray(outputs, dtype=np.float32)
    labels = np.asarray(labels, dtype=np.float32)
    training_masks = np.asarray(training_masks, dtype=np.float32)
    assert outputs.shape == (B, C, H, W)

    bf16 = _np_bf16()
    nc = _get_nc()
    in_maps = [
        {
            "x": np.ascontiguousarray(outputs[b]).astype(bf16),
            "g": np.ascontiguousarray(labels[b]).astype(bf16),
            "m": np.ascontiguousarray(training_masks[b]).astype(bf16),
        }
        for b in range(B)
    ]
    r = None
    for attempt in range(3):
        try:
            r = run_bass_kernel_spmd(
                nc, in_maps, list(range(NCORES)),
                trace=_CACHE.get("trace", False),
            )
            break
        except Exception:
            if attempt == 2:
                raise
            _CACHE.pop("nc", None)
            nc = _get_nc()
    _CACHE["last_result"] = r
    res_list = [r.results[b]["res"] for b in range(B)]
    out = _combine(res_list)
    if out is None:
        # OHEM threshold is not the minimum negative score -> exact host path
        out = _numpy_reference(outputs, labels, training_masks)
    return out
